# revision 12
# baseline (speedup 1.0000x reference)
"""GQA multi-head attention (B=4,T=2048,E=1024, 8 q-heads / 4 kv-heads, RoPE,
causal) on 8 TRN2 NeuronCores.

Sharding: data-parallel over batch (4) x tensor-parallel over head groups (2).
Core c = 2*b + g handles batch b with q-heads [4g..4g+4) / kv-heads [2g..2g+2).
Each core computes a partial o_proj output; the host sums the two partials per
batch (the all-reduce of the column-sharded o_proj).

Fused single-pass pipeline over 4 query quads (512 t columns each):
  per quad tq: QKV matmuls + RoPE + PE-transpose into qkT [d, t], previous
  quad's o_proj blocks as PE filler, then causal score column-stripes per head
  (k-stationary x q-cols, transposed [s, t] layout) computed in PAIRS into a
  2-bank psum so a single ACT exp drains 1024 columns at a time (halves the
  exp instruction count; ACT per-op overhead is ~400ns).  GpSimd masks the
  diagonal blocks, PV runs with a ones-column appended to v (psum [t, d+1]
  carries the softmax denominator), DVE normalizes, PE transposes back.
  QKV and o_proj accumulate in [128,512] half-psums sharing one 2-slot ring
  (2 banks total) so the score pairs can have 4 banks.  PSUM drains are split
  across ACT and DVE so neither engine becomes the pacer.  The Tile
  scheduler's per-engine ready-heaps interleave next-quad QKV and prior-quad
  o_proj matmuls into the stretches where PV waits on ACT exp.
"""

import numpy as np
import ml_dtypes
from contextlib import ExitStack

import concourse.bass as bass
import concourse.tile as tile
from concourse import bacc, mybir
from concourse.bass import ts, ds
from concourse.bass_utils import run_bass_kernel_spmd
from concourse.masks import make_identity

BF16 = mybir.dt.bfloat16
F32 = mybir.dt.float32

B, T, E = 4, 2048, 1024
HQ, HKV, D = 8, 4, 128
G = HQ // HKV          # 2 q heads per kv head
NGRP = 2               # tensor-parallel head groups
QH = HQ // NGRP        # 4 local q heads
KH = HKV // NGRP       # 2 local kv heads
NH = QH + KH           # 6 rope'd heads
FQ = QH * D            # 512 local q features
FK = KH * D            # 256 local kv features
FA = FQ + 2 * FK       # 1024 fused qkv features
EO = E // 128          # 8 contraction chunks
NTB = T // 128         # 16 t/s blocks of 128
NQUAD = T // 512       # 4 query quads of 512
DH = D // 2            # 64 rope half
SCALE = 1.0 / float(np.sqrt(D))
NCORES = 8
EXP = mybir.ActivationFunctionType.Exp


def _body(tc, ctx, io):
    nc = tc.nc
    xT, wqkvT, woT, cosN, sinN, maskT, out = io

    const = ctx.enter_context(tc.tile_pool(name="const", bufs=1))
    pers = ctx.enter_context(tc.tile_pool(name="pers", bufs=1))
    tmp = ctx.enter_context(tc.tile_pool(name="tmp", bufs=3))
    xcp = ctx.enter_context(tc.tile_pool(name="xcp", bufs=2))
    attnp = ctx.enter_context(tc.tile_pool(name="attnp", bufs=3))
    outp = ctx.enter_context(tc.tile_pool(name="outp", bufs=2))
    # PSUM budget (8 banks): half 2x1 (qkv + o_proj half-accs in one ring),
    # score pairs 2x2, small 2x1 (pv acc + transposes in one ring).
    psHalf = ctx.enter_context(tc.tile_pool(name="psHalf", bufs=2, space="PSUM"))
    psSc = ctx.enter_context(tc.tile_pool(name="psSc", bufs=2, space="PSUM"))
    psSm = ctx.enter_context(tc.tile_pool(name="psSm", bufs=2, space="PSUM"))

    wqkv_sb = pers.tile([128, EO, FA], BF16)
    wo_sb = pers.tile([128, FQ // 128, E], BF16)
    cos_sb = const.tile([128, NTB, DH], BF16)
    sin_sb = const.tile([128, NTB, DH], BF16)
    mask_sb = const.tile([128, 128], BF16)
    ident = const.tile([128, 128], BF16)
    make_identity(nc, ident[:])

    qkT_sb = pers.tile([128, NH, T], BF16)     # q heads 0..3, k heads 4..5
    vaug_sb = pers.tile([128, NTB, KH, D + 1], BF16)
    nc.vector.memset(vaug_sb[:, :, :, D : D + 1], 1.0)
    attnoutT_sb = pers.tile([128, QH, T], BF16)

    # per-(quad, head) attention stripe: 16 s-blocks x 512 t-cols, contiguous
    # so a pair exp can write 1024 columns in one ACT op
    attn_stripe = [None] * QH

    def stripe_pair(h, sc0, tq, ath):
        kv = h // G
        pss = psSc.tile([128, 1024], F32, tag="sc", name="pss")
        for k in range(2):
            sc = sc0 + k
            j0 = max(0, sc - 4 * tq) * 128
            w = 512 - j0
            nc.tensor.matmul(
                pss[:, ds(k * 512 + j0, w)],
                qkT_sb[:, QH + kv, ts(sc, 128)],
                qkT_sb[:, h, ds(tq * 512 + j0, w)],
                start=True, stop=True)
        # one exp drains both halves; cols left of a diagonal block hold
        # stale psum and are never read downstream
        nc.scalar.activation(ath[:, ds(sc0 * 512, 1024)], pss[:, 0:1024],
                             EXP, scale=SCALE)
        for k in range(2):
            sc = sc0 + k
            if sc >= 4 * tq:
                jd = (sc - 4 * tq) * 128
                nc.gpsimd.tensor_mul(
                    ath[:, ds(sc * 512 + jd, 128)],
                    ath[:, ds(sc * 512 + jd, 128)], mask_sb[:])

    def pv_pair(h, tb0, tq, aostage):
        """PV for t-blocks tb0, tb0+1 into one 1-bank psum pair; the softmax
        reciprocal+normalize run batched (one DVE op each per pair)."""
        kv = h // G
        i0 = tb0 - 4 * tq
        ath = attn_stripe[h]
        po2 = psSm.tile([128, 2, D + 1], F32, tag="psB", name="po2")
        for j in range(2):
            tb = tb0 + j
            for sc in range(tb + 1):
                nc.tensor.matmul(
                    po2[:, j, :], ath[:, ds(sc * 512 + (i0 + j) * 128, 128)],
                    vaug_sb[:, sc, kv, :],
                    start=(sc == 0), stop=(sc == tb))
        rs2 = tmp.tile([128, 2], F32, tag="rs", name="rs")
        nc.vector.reciprocal(rs2[:], po2[:, :, D])
        nc.vector.tensor_mul(
            aostage[:, i0 : i0 + 2, :], po2[:, :, 0:D],
            rs2[:, :, None].to_broadcast((128, 2, D)))

    def oproj_block(tb):
        ob = outp.tile([128, E], BF16, tag="ob", name="ob")
        psoA = psHalf.tile([128, 512], F32, tag="half", name="psoA")
        psoB = psHalf.tile([128, 512], F32, tag="half", name="psoB")
        for fo in range(QH):
            nc.tensor.matmul(
                psoA[:], attnoutT_sb[:, fo, ts(tb, 128)],
                wo_sb[:, fo, ds(0, 512)],
                start=(fo == 0), stop=(fo == QH - 1))
        for fo in range(QH):
            nc.tensor.matmul(
                psoB[:], attnoutT_sb[:, fo, ts(tb, 128)],
                wo_sb[:, fo, ds(512, 512)],
                start=(fo == 0), stop=(fo == QH - 1))
        # split the psum drain across DVE/ACT so neither becomes the pacer
        nc.vector.tensor_copy(ob[:, 0:512], psoA[:])
        nc.scalar.copy(ob[:, ds(512, 512)], psoB[:])
        nc.sync.dma_start(out[ds(tb * 128, 128), :], ob[:])

    first = True
    for tq in range(NQUAD):
        # ---- QKV for quad tq ----
        xc = xcp.tile([128, EO, 512], BF16, tag="xc", name=f"xc{tq}")
        for eo in range(EO):
            nc.sync.dma_start(xc[:, eo, :], xT[ds(eo * 128, 128), ts(tq, 512)])
            if first:
                nc.sync.dma_start(wqkv_sb[:, eo, :], wqkvT[ds(eo * 128, 128), :])
                if eo == 2:
                    # rope tables ride behind the first few contraction chunks
                    # so the startup matmuls are not queued behind them
                    nc.sync.dma_start(
                        cos_sb[:], cosN.rearrange("p (tb i) -> p tb i", i=DH))
                    nc.sync.dma_start(
                        sin_sb[:], sinN.rearrange("p (tb i) -> p tb i", i=DH))
                    nc.sync.dma_start(mask_sb[:], maskT[:])

        for tbl in range(4):
            tb = tq * 4 + tbl
            pqA = psHalf.tile([128, 512], F32, tag="half", name="pqA")
            pqB = psHalf.tile([128, 512], F32, tag="half", name="pqB")
            for eo in range(EO):
                nc.tensor.matmul(
                    pqA[:], xc[:, eo, ts(tbl, 128)],
                    wqkv_sb[:, eo, ds(0, 512)],
                    start=(eo == 0), stop=(eo == EO - 1))
                nc.tensor.matmul(
                    pqB[:], xc[:, eo, ts(tbl, 128)],
                    wqkv_sb[:, eo, ds(512, 512)],
                    start=(eo == 0), stop=(eo == EO - 1))
            # rope'd heads drain via ACT+DVE; v goes straight to vaug on DVE
            raw = tmp.tile([128, NH * D], BF16, tag="raw", name="raw", bufs=3)
            nc.scalar.copy(raw[:, 0:512], pqA[:])
            nc.vector.tensor_copy(raw[:, ds(512, 256)], pqB[:, 0:256])
            nc.vector.tensor_copy(
                vaug_sb[:, tb, :, 0:D],
                pqB[:, ds(256, KH * D)].rearrange("p (k d) -> p k d", d=D))
            raw3 = raw.rearrange("p (h d) -> p h d", d=D)
            rot = tmp.tile([128, NH, D], BF16, tag="rot", name="rot", bufs=2)
            c6 = cos_sb[:, tb, None, :].to_broadcast((128, NH, DH))
            s6 = sin_sb[:, tb, None, :].to_broadcast((128, NH, DH))
            x1 = raw3[:, 0:NH, 0:DH]
            x2 = raw3[:, 0:NH, DH:D]
            t1 = tmp.tile([128, NH, DH], BF16, tag="t1", name="t1", bufs=2)
            o1 = rot[:, :, 0:DH]
            o2 = rot[:, :, DH:D]
            # the x2 products run on the otherwise-idle GpSimd so DVE (the
            # pacer of QKV windows) only carries 4 of the 6 rope ops
            t2 = tmp.tile([128, NH, DH], BF16, tag="t2", name="t2", bufs=2)
            nc.gpsimd.tensor_mul(t1[:], x2, s6)
            nc.gpsimd.tensor_mul(t2[:], x2, c6)
            nc.vector.tensor_mul(o1, x1, c6)
            nc.vector.tensor_mul(o2, x1, s6)
            nc.vector.tensor_sub(o1, o1, t1[:])
            nc.vector.tensor_add(o2, o2, t2[:])
            # transpose rope'd heads into [d, t] layout, one batched copy
            # (drained on ACT, which is idle during QKV windows)
            ptq = psSm.tile([128, NH, 128], BF16, tag="psB", name="ptq")
            for hh in range(NH):
                nc.tensor.transpose(ptq[:, hh, :], rot[:, hh, :], ident[:])
            nc.scalar.copy(qkT_sb[:, :, ts(tb, 128)], ptq[:])
        # ---- o_proj of quad tq-1 rides along as PE filler ----
        if tq > 0:
            for tbl in range(4):
                oproj_block((tq - 1) * 4 + tbl)
        if first:
            for fo in range(FQ // 128):
                nc.sync.dma_start(wo_sb[:, fo, :], woT[ds(fo * 128, 128), :])
        first = False

        # ---- score stripes + PV for quad tq ----
        nsc = 4 * tq + 4
        for h in range(QH):
            attn_stripe[h] = attnp.tile([128, NTB * 512], BF16, tag="ath",
                                        name=f"ath{h}_{tq}")
        for sc0 in range(0, nsc, 2):
            stripe_pair(0, sc0, tq, attn_stripe[0])
        for h in range(QH):
            if h + 1 < QH:
                # next head's stripe interleaves with this head's PV so PE
                # has non-ACT-gated filler while exp paces the stripes
                for sc0 in range(0, nsc, 2):
                    stripe_pair(h + 1, sc0, tq, attn_stripe[h + 1])
            aostage = tmp.tile([128, 4, D], BF16, tag="aost", name="aost",
                               bufs=2)
            if h < QH - 1:
                # batched attnout transpose: 4 blocks per PE-transpose
                # group, one 512-col DVE copy instead of four 128-col ones
                pv_pair(h, tq * 4, tq, aostage)
                pv_pair(h, tq * 4 + 2, tq, aostage)
                pt2 = psSm.tile([128, 4, 128], BF16, tag="psB", name="pt2b")
                for j in range(4):
                    nc.tensor.transpose(pt2[:, j, :], aostage[:, j, :],
                                        ident[:])
                nc.vector.tensor_copy(
                    attnoutT_sb[:, h, ds(tq * 512, 512)], pt2[:])
            else:
                # last head: per-pair transpose so o_proj of this quad can
                # chase the PV pairs
                for p in range(2):
                    pv_pair(h, tq * 4 + 2 * p, tq, aostage)
                    pt2 = psSm.tile([128, 2, 128], BF16, tag="psB",
                                    name="pt2p")
                    for j in range(2):
                        nc.tensor.transpose(pt2[:, j, :],
                                            aostage[:, 2 * p + j, :],
                                            ident[:])
                    nc.vector.tensor_copy(
                        attnoutT_sb[:, h, ds(tq * 512 + p * 256, 256)],
                        pt2[:])
    for i in range(4):
        oproj_block((NQUAD - 1) * 4 + i)


def build():
    nc = bacc.Bacc("TRN2", target_bir_lowering=False, debug=False,
                   enable_asserts=False)
    xT = nc.dram_tensor("xT", [E, T], BF16, kind="ExternalInput").ap()
    wqkvT = nc.dram_tensor("wqkvT", [E, FA], BF16, kind="ExternalInput").ap()
    woT = nc.dram_tensor("woT", [FQ, E], BF16, kind="ExternalInput").ap()
    cosN = nc.dram_tensor("cosN", [128, NTB * DH], BF16, kind="ExternalInput").ap()
    sinN = nc.dram_tensor("sinN", [128, NTB * DH], BF16, kind="ExternalInput").ap()
    maskT = nc.dram_tensor("maskT", [128, 128], BF16, kind="ExternalInput").ap()
    out = nc.dram_tensor("out", [T, E], BF16, kind="ExternalOutput").ap()
    io = (xT, wqkvT, woT, cosN, sinN, maskT, out)
    with tile.TileContext(nc) as tc, ExitStack() as ctx:
        _body(tc, ctx, io)
    nc.compile()
    return nc


_NC_CACHE = None


def _get_nc():
    global _NC_CACHE
    if _NC_CACHE is None:
        _NC_CACHE = build()
    return _NC_CACHE


def _bf16(a):
    return np.ascontiguousarray(a).astype(ml_dtypes.bfloat16)


def make_in_maps(x, Wq, Wk, Wv, Wo):
    inv_freq = (1.0 / (10000.0 ** (2.0 * np.arange(DH, dtype=np.float32) / D)))
    theta = np.arange(T, dtype=np.float32)[:, None] * inv_freq[None, :]
    # pre-tiled [128, NTB*DH]: row p holds cos(theta[tb*128+p, :]) for each tb
    cosN = _bf16(np.cos(theta).reshape(NTB, 128, DH).transpose(1, 0, 2).reshape(128, NTB * DH))
    sinN = _bf16(np.sin(theta).reshape(NTB, 128, DH).transpose(1, 0, 2).reshape(128, NTB * DH))
    ls = np.arange(128)
    maskT = _bf16((ls[:, None] <= ls[None, :]).astype(np.float32))  # s<=t valid
    in_maps = []
    for c in range(NCORES):
        b, g = c // NGRP, c % NGRP
        wq = Wq[g * FQ:(g + 1) * FQ, :]      # [512, 1024]
        wk = Wk[g * FK:(g + 1) * FK, :]      # [256, 1024]
        wv = Wv[g * FK:(g + 1) * FK, :]      # [256, 1024]
        wqkv = np.concatenate([wq, wk, wv], axis=0)   # [1024, 1024]
        in_maps.append({
            "xT": _bf16(x[b].T),
            "wqkvT": _bf16(wqkv.T),
            "woT": _bf16(Wo[:, g * FQ:(g + 1) * FQ].T),
            "cosN": cosN, "sinN": sinN, "maskT": maskT,
        })
    return in_maps


def kernel(x, Wq, Wk, Wv, Wo, _trace=False):
    nc = _get_nc()
    in_maps = make_in_maps(np.asarray(x, dtype=np.float32),
                           np.asarray(Wq, dtype=np.float32),
                           np.asarray(Wk, dtype=np.float32),
                           np.asarray(Wv, dtype=np.float32),
                           np.asarray(Wo, dtype=np.float32))
    # The first execution after a fresh NEFF load occasionally hits a
    # transient NRT_EXEC_UNIT_UNRECOVERABLE; a retry recovers.
    last_err = None
    for _attempt in range(3):
        try:
            res = run_bass_kernel_spmd(nc, in_maps,
                                       core_ids=list(range(NCORES)),
                                       trace=_trace)
            break
        except Exception as e:  # noqa: BLE001
            last_err = e
    else:
        raise last_err
    outs = [r["out"].astype(np.float32) for r in res.results]
    full = np.stack([outs[2 * b] + outs[2 * b + 1] for b in range(B)], axis=0)
    if _trace:
        kernel.last_exec_time_ns = res.exec_time_ns
        kernel.last_results = res
    return full


# revision 15
# speedup vs baseline: 1.1044x; 1.1044x over previous
"""GQA multi-head attention (B=4,T=2048,E=1024, 8 q-heads / 4 kv-heads, RoPE,
causal) on 8 TRN2 NeuronCores.

Sharding: data-parallel over batch (4) x tensor-parallel over head groups (2).
Core c = 2*b + g handles batch b with q-heads [4g..4g+4) / kv-heads [2g..2g+2).
Each core computes a partial o_proj output; the host sums the two partials per
batch (the all-reduce of the column-sharded o_proj).

Fused single-pass pipeline over 4 query quads (512 t columns each):
  per quad tq: QKV matmuls + RoPE + PE-transpose into qkT [d, t], previous
  quad's o_proj blocks as PE filler, then causal score column-stripes per head
  (k-stationary x q-cols, transposed [s, t] layout) computed in PAIRS into a
  2-bank psum so a single ACT exp drains 1024 columns at a time (halves the
  exp instruction count; ACT per-op overhead is ~400ns).  GpSimd masks the
  diagonal blocks, PV runs with a ones-column appended to v (psum [t, d+1]
  carries the softmax denominator), DVE normalizes, PE transposes back.
  QKV and o_proj accumulate in [128,512] half-psums sharing one 2-slot ring
  (2 banks total) so the score pairs can have 4 banks.  PSUM drains are split
  across ACT and DVE so neither engine becomes the pacer.  The Tile
  scheduler's per-engine ready-heaps interleave next-quad QKV and prior-quad
  o_proj matmuls into the stretches where PV waits on ACT exp.
"""

import numpy as np
import ml_dtypes
from contextlib import ExitStack

import concourse.bass as bass
import concourse.tile as tile
from concourse import bacc, mybir
from concourse.bass import ts, ds
from concourse.bass_utils import run_bass_kernel_spmd
from concourse.masks import make_identity

BF16 = mybir.dt.bfloat16
F32 = mybir.dt.float32

B, T, E = 4, 2048, 1024
HQ, HKV, D = 8, 4, 128
G = HQ // HKV          # 2 q heads per kv head
NGRP = 2               # tensor-parallel head groups
QH = HQ // NGRP        # 4 local q heads
KH = HKV // NGRP       # 2 local kv heads
NH = QH + KH           # 6 rope'd heads
FQ = QH * D            # 512 local q features
FK = KH * D            # 256 local kv features
FA = FQ + 2 * FK       # 1024 fused qkv features
EO = E // 128          # 8 contraction chunks
NTB = T // 128         # 16 t/s blocks of 128
NQUAD = T // 512       # 4 query quads of 512
DH = D // 2            # 64 rope half
SCALE = 1.0 / float(np.sqrt(D))
NCORES = 8
EXP = mybir.ActivationFunctionType.Exp


def _body(tc, ctx, io):
    nc = tc.nc
    xT, wqkvT, woT, cosN, sinN, maskT, out = io

    const = ctx.enter_context(tc.tile_pool(name="const", bufs=1))
    pers = ctx.enter_context(tc.tile_pool(name="pers", bufs=1))
    tmp = ctx.enter_context(tc.tile_pool(name="tmp", bufs=3))
    xcp = ctx.enter_context(tc.tile_pool(name="xcp", bufs=2))
    attnp = ctx.enter_context(tc.tile_pool(name="attnp", bufs=3))
    outp = ctx.enter_context(tc.tile_pool(name="outp", bufs=2))
    # PSUM budget (8 banks): half 2x1 (qkv + o_proj half-accs in one ring),
    # score pairs 2x2, small 2x1 (pv acc + transposes in one ring).
    psHalf = ctx.enter_context(tc.tile_pool(name="psHalf", bufs=2, space="PSUM"))
    psSc = ctx.enter_context(tc.tile_pool(name="psSc", bufs=2, space="PSUM"))
    psSm = ctx.enter_context(tc.tile_pool(name="psSm", bufs=2, space="PSUM"))

    wqkv_sb = pers.tile([128, EO, FA], BF16)
    wo_sb = pers.tile([128, FQ // 128, E], BF16)
    cos_sb = const.tile([128, NTB, DH], BF16)
    sin_sb = const.tile([128, NTB, DH], BF16)
    mask_sb = const.tile([128, 128], BF16)
    ident = const.tile([128, 128], BF16)
    make_identity(nc, ident[:])

    qkT_sb = pers.tile([128, NH, T], BF16)     # q heads 0..3, k heads 4..5
    vaug_sb = pers.tile([128, NTB, KH, D + 1], BF16)
    nc.vector.memset(vaug_sb[:, :, :, D : D + 1], 1.0)
    attnoutT_sb = pers.tile([128, QH, T], BF16)

    # per-(quad, head) attention stripe: 16 s-blocks x 512 t-cols, contiguous
    # so a pair exp can write 1024 columns in one ACT op
    attn_stripe = [None] * QH

    def stripe_pair(h, sc0, tq, ath):
        kv = h // G
        pss = psSc.tile([128, 1024], F32, tag="sc", name="pss")
        for k in range(2):
            sc = sc0 + k
            j0 = max(0, sc - 4 * tq) * 128
            w = 512 - j0
            nc.tensor.matmul(
                pss[:, ds(k * 512 + j0, w)],
                qkT_sb[:, QH + kv, ts(sc, 128)],
                qkT_sb[:, h, ds(tq * 512 + j0, w)],
                start=True, stop=True)
        # one exp drains both halves; cols left of a diagonal block hold
        # stale psum and are never read downstream
        nc.scalar.activation(ath[:, ds(sc0 * 512, 1024)], pss[:, 0:1024],
                             EXP, scale=SCALE)
        for k in range(2):
            sc = sc0 + k
            if sc >= 4 * tq:
                jd = (sc - 4 * tq) * 128
                nc.gpsimd.tensor_mul(
                    ath[:, ds(sc * 512 + jd, 128)],
                    ath[:, ds(sc * 512 + jd, 128)], mask_sb[:])

    def pv_pair(h, tb0, tq, aostage):
        """PV for t-blocks tb0, tb0+1 into one 1-bank psum pair; the softmax
        reciprocal+normalize run batched (one DVE op each per pair)."""
        kv = h // G
        i0 = tb0 - 4 * tq
        ath = attn_stripe[h]
        po2 = psSm.tile([128, 2, D + 1], F32, tag="psB", name="po2")
        for j in range(2):
            tb = tb0 + j
            for sc in range(tb + 1):
                nc.tensor.matmul(
                    po2[:, j, :], ath[:, ds(sc * 512 + (i0 + j) * 128, 128)],
                    vaug_sb[:, sc, kv, :],
                    start=(sc == 0), stop=(sc == tb))
        rs2 = tmp.tile([128, 2], F32, tag="rs", name="rs")
        nc.vector.reciprocal(rs2[:], po2[:, :, D])
        nc.vector.tensor_mul(
            aostage[:, i0 : i0 + 2, :], po2[:, :, 0:D],
            rs2[:, :, None].to_broadcast((128, 2, D)))

    def oproj_block(tb):
        ob = outp.tile([128, E], BF16, tag="ob", name="ob")
        psoA = psHalf.tile([128, 512], F32, tag="half", name="psoA")
        psoB = psHalf.tile([128, 512], F32, tag="half", name="psoB")
        for fo in range(QH):
            nc.tensor.matmul(
                psoA[:], attnoutT_sb[:, fo, ts(tb, 128)],
                wo_sb[:, fo, ds(0, 512)],
                start=(fo == 0), stop=(fo == QH - 1))
        for fo in range(QH):
            nc.tensor.matmul(
                psoB[:], attnoutT_sb[:, fo, ts(tb, 128)],
                wo_sb[:, fo, ds(512, 512)],
                start=(fo == 0), stop=(fo == QH - 1))
        # split the psum drain across DVE/ACT so neither becomes the pacer
        nc.vector.tensor_copy(ob[:, 0:512], psoA[:])
        nc.scalar.copy(ob[:, ds(512, 512)], psoB[:])
        nc.sync.dma_start(out[ds(tb * 128, 128), :], ob[:])

    first = True
    for tq in range(NQUAD):
        # ---- QKV for quad tq ----
        xc = xcp.tile([128, EO, 512], BF16, tag="xc", name=f"xc{tq}")
        for eo in range(EO):
            nc.sync.dma_start(xc[:, eo, :], xT[ds(eo * 128, 128), ts(tq, 512)])
            if first:
                nc.sync.dma_start(wqkv_sb[:, eo, :], wqkvT[ds(eo * 128, 128), :])
                if eo == 2:
                    # rope tables ride behind the first few contraction chunks
                    # so the startup matmuls are not queued behind them
                    nc.sync.dma_start(
                        cos_sb[:], cosN.rearrange("p (tb i) -> p tb i", i=DH))
                    nc.sync.dma_start(
                        sin_sb[:], sinN.rearrange("p (tb i) -> p tb i", i=DH))
                    nc.sync.dma_start(mask_sb[:], maskT[:])

        def qkv_mms(dstA, dstB, tbl, eo):
            nc.tensor.matmul(
                dstA, xc[:, eo, ts(tbl, 128)], wqkv_sb[:, eo, ds(0, 512)],
                start=(eo == 0), stop=(eo == EO - 1))
            nc.tensor.matmul(
                dstB, xc[:, eo, ts(tbl, 128)], wqkv_sb[:, eo, ds(512, 512)],
                start=(eo == 0), stop=(eo == EO - 1))

        if first:
            # startup: accumulate t-blocks 0-2 eo-major in parallel (score
            # psum banks are still idle), so each arriving x/weight chunk
            # feeds 6 matmuls while the initial DMAs stream in
            g01 = psSc.tile([128, 1024], F32, tag="sc", name="pq01")
            g11 = psSc.tile([128, 1024], F32, tag="sc", name="pq11")
            gA2 = psHalf.tile([128, 512], F32, tag="half", name="pqA2")
            gB2 = psHalf.tile([128, 512], F32, tag="half", name="pqB2")
            for eo in range(EO):
                qkv_mms(g01[:, 0:512], g01[:, ds(512, 512)], 0, eo)
                qkv_mms(g11[:, 0:512], g11[:, ds(512, 512)], 1, eo)
                qkv_mms(gA2[:], gB2[:], 2, eo)
            plan = [(0, g01), (1, g11), (2, (gA2, gB2)), (3, None)]
        else:
            plan = [(t, None) for t in range(4)]
        for tbl, src in plan:
            tb = tq * 4 + tbl
            if src is None:
                tA = psHalf.tile([128, 512], F32, tag="half", name="pqA")
                tB = psHalf.tile([128, 512], F32, tag="half", name="pqB")
                for eo in range(EO):
                    qkv_mms(tA[:], tB[:], tbl, eo)
                src = (tA, tB)
            if isinstance(src, tuple):
                tA, tB = src
                a_src = tA[:]
                bk_src = tB[:, 0:256]
                bv_src = tB[:, ds(256, KH * D)]
            else:
                a_src = src[:, 0:512]
                bk_src = src[:, ds(512, 256)]
                bv_src = src[:, ds(768, KH * D)]
            # rope'd heads drain via ACT+DVE; v goes straight to vaug on DVE
            raw = tmp.tile([128, NH * D], BF16, tag="raw", name="raw", bufs=3)
            nc.scalar.copy(raw[:, 0:512], a_src)
            nc.vector.tensor_copy(raw[:, ds(512, 256)], bk_src)
            nc.vector.tensor_copy(
                vaug_sb[:, tb, :, 0:D],
                bv_src.rearrange("p (k d) -> p k d", d=D))
            raw3 = raw.rearrange("p (h d) -> p h d", d=D)
            rot = tmp.tile([128, NH, D], BF16, tag="rot", name="rot", bufs=2)
            c6 = cos_sb[:, tb, None, :].to_broadcast((128, NH, DH))
            s6 = sin_sb[:, tb, None, :].to_broadcast((128, NH, DH))
            x1 = raw3[:, 0:NH, 0:DH]
            x2 = raw3[:, 0:NH, DH:D]
            t1 = tmp.tile([128, NH, DH], BF16, tag="t1", name="t1", bufs=2)
            o1 = rot[:, :, 0:DH]
            o2 = rot[:, :, DH:D]
            nc.vector.tensor_mul(o1, x1, c6)
            nc.vector.tensor_mul(t1[:], x2, s6)
            nc.vector.tensor_sub(o1, o1, t1[:])
            nc.vector.tensor_mul(o2, x1, s6)
            nc.vector.tensor_mul(t1[:], x2, c6)
            nc.vector.tensor_add(o2, o2, t1[:])
            # transpose rope'd heads into [d, t] layout, one batched copy
            ptq = psSm.tile([128, NH, 128], BF16, tag="psB", name="ptq")
            for hh in range(NH):
                nc.tensor.transpose(ptq[:, hh, :], rot[:, hh, :], ident[:])
            nc.vector.tensor_copy(qkT_sb[:, :, ts(tb, 128)], ptq[:])
        # ---- o_proj of quad tq-1 rides along as PE filler ----
        if tq > 0:
            for tbl in range(4):
                oproj_block((tq - 1) * 4 + tbl)
        if first:
            for fo in range(FQ // 128):
                nc.sync.dma_start(wo_sb[:, fo, :], woT[ds(fo * 128, 128), :])
        first = False

        # ---- score stripes + PV for quad tq ----
        nsc = 4 * tq + 4
        for h in range(QH):
            attn_stripe[h] = attnp.tile([128, NTB * 512], BF16, tag="ath",
                                        name=f"ath{h}_{tq}")
        for sc0 in range(0, nsc, 2):
            stripe_pair(0, sc0, tq, attn_stripe[0])
        for h in range(QH):
            if h + 1 < QH:
                # next head's stripe interleaves with this head's PV so PE
                # has non-ACT-gated filler while exp paces the stripes
                for sc0 in range(0, nsc, 2):
                    stripe_pair(h + 1, sc0, tq, attn_stripe[h + 1])
            aostage = tmp.tile([128, 4, D], BF16, tag="aost", name="aost",
                               bufs=2)
            if h < QH - 1:
                # batched attnout transpose: 4 blocks per PE-transpose
                # group, one 512-col DVE copy instead of four 128-col ones
                pv_pair(h, tq * 4, tq, aostage)
                pv_pair(h, tq * 4 + 2, tq, aostage)
                pt2 = psSm.tile([128, 4, 128], BF16, tag="psB", name="pt2b")
                for j in range(4):
                    nc.tensor.transpose(pt2[:, j, :], aostage[:, j, :],
                                        ident[:])
                nc.vector.tensor_copy(
                    attnoutT_sb[:, h, ds(tq * 512, 512)], pt2[:])
            else:
                # last head: per-pair transpose so o_proj of this quad can
                # chase the PV pairs
                for p in range(2):
                    pv_pair(h, tq * 4 + 2 * p, tq, aostage)
                    pt2 = psSm.tile([128, 2, 128], BF16, tag="psB",
                                    name="pt2p")
                    for j in range(2):
                        nc.tensor.transpose(pt2[:, j, :],
                                            aostage[:, 2 * p + j, :],
                                            ident[:])
                    nc.vector.tensor_copy(
                        attnoutT_sb[:, h, ds(tq * 512 + p * 256, 256)],
                        pt2[:])
    for i in range(4):
        oproj_block((NQUAD - 1) * 4 + i)


def build():
    nc = bacc.Bacc("TRN2", target_bir_lowering=False, debug=False,
                   enable_asserts=False)
    xT = nc.dram_tensor("xT", [E, T], BF16, kind="ExternalInput").ap()
    wqkvT = nc.dram_tensor("wqkvT", [E, FA], BF16, kind="ExternalInput").ap()
    woT = nc.dram_tensor("woT", [FQ, E], BF16, kind="ExternalInput").ap()
    cosN = nc.dram_tensor("cosN", [128, NTB * DH], BF16, kind="ExternalInput").ap()
    sinN = nc.dram_tensor("sinN", [128, NTB * DH], BF16, kind="ExternalInput").ap()
    maskT = nc.dram_tensor("maskT", [128, 128], BF16, kind="ExternalInput").ap()
    out = nc.dram_tensor("out", [T, E], BF16, kind="ExternalOutput").ap()
    io = (xT, wqkvT, woT, cosN, sinN, maskT, out)
    with tile.TileContext(nc) as tc, ExitStack() as ctx:
        _body(tc, ctx, io)
    nc.compile()
    return nc


_NC_CACHE = None


def _get_nc():
    global _NC_CACHE
    if _NC_CACHE is None:
        _NC_CACHE = build()
    return _NC_CACHE


def _bf16(a):
    return np.ascontiguousarray(a).astype(ml_dtypes.bfloat16)


def make_in_maps(x, Wq, Wk, Wv, Wo):
    inv_freq = (1.0 / (10000.0 ** (2.0 * np.arange(DH, dtype=np.float32) / D)))
    theta = np.arange(T, dtype=np.float32)[:, None] * inv_freq[None, :]
    # pre-tiled [128, NTB*DH]: row p holds cos(theta[tb*128+p, :]) for each tb
    cosN = _bf16(np.cos(theta).reshape(NTB, 128, DH).transpose(1, 0, 2).reshape(128, NTB * DH))
    sinN = _bf16(np.sin(theta).reshape(NTB, 128, DH).transpose(1, 0, 2).reshape(128, NTB * DH))
    ls = np.arange(128)
    maskT = _bf16((ls[:, None] <= ls[None, :]).astype(np.float32))  # s<=t valid
    in_maps = []
    for c in range(NCORES):
        b, g = c // NGRP, c % NGRP
        wq = Wq[g * FQ:(g + 1) * FQ, :]      # [512, 1024]
        wk = Wk[g * FK:(g + 1) * FK, :]      # [256, 1024]
        wv = Wv[g * FK:(g + 1) * FK, :]      # [256, 1024]
        wqkv = np.concatenate([wq, wk, wv], axis=0)   # [1024, 1024]
        in_maps.append({
            "xT": _bf16(x[b].T),
            "wqkvT": _bf16(wqkv.T),
            "woT": _bf16(Wo[:, g * FQ:(g + 1) * FQ].T),
            "cosN": cosN, "sinN": sinN, "maskT": maskT,
        })
    return in_maps


def kernel(x, Wq, Wk, Wv, Wo, _trace=False):
    nc = _get_nc()
    in_maps = make_in_maps(np.asarray(x, dtype=np.float32),
                           np.asarray(Wq, dtype=np.float32),
                           np.asarray(Wk, dtype=np.float32),
                           np.asarray(Wv, dtype=np.float32),
                           np.asarray(Wo, dtype=np.float32))
    # The first execution after a fresh NEFF load occasionally hits a
    # transient NRT_EXEC_UNIT_UNRECOVERABLE; a retry recovers.
    last_err = None
    for _attempt in range(3):
        try:
            res = run_bass_kernel_spmd(nc, in_maps,
                                       core_ids=list(range(NCORES)),
                                       trace=_trace)
            break
        except Exception as e:  # noqa: BLE001
            last_err = e
    else:
        raise last_err
    outs = [r["out"].astype(np.float32) for r in res.results]
    full = np.stack([outs[2 * b] + outs[2 * b + 1] for b in range(B)], axis=0)
    if _trace:
        kernel.last_exec_time_ns = res.exec_time_ns
        kernel.last_results = res
    return full
